# revision 1
# baseline (speedup 1.0000x reference)
"""Dilated-attention transformer block kernel for TRN2, 8-core SPMD.

Sharding: (batch b in {0,1}) x (sequence chunk c in {0..3}) -> 8 cores.
Each core computes the full block for its 512 tokens, with a 512-token
halo for K/V. Dilation=2 is handled by parity-grouping the sequence
(even tokens then odd tokens); within a parity the mask is a causal
local window of 256 parity-steps. Attention is processed per parity to
halve SBUF residency.
"""
import numpy as np
import concourse.bass as bass
from concourse import bacc
import concourse.mybir as mybir
from concourse.tile import TileContext
from concourse.bass_utils import run_bass_kernel_spmd
from concourse.masks import make_identity

dt = mybir.dt
F32, F32R = dt.float32, dt.float32r
AF = mybir.ActivationFunctionType
OP = mybir.AluOpType

B, L, D, H, HD = 2, 2048, 1024, 16, 64
SCALE = 1.0 / float(np.sqrt(HD))
NEG = -10000.0


def build(reps: int = 1):
    nc = bacc.Bacc(None, target_bir_lowering=False)
    xg_d = nc.declare_dram_parameter("xg", [1024, 1024], F32, isOutput=False)
    wqkv_d = nc.declare_dram_parameter("wqkv", [1024, 3072], F32R, isOutput=False)
    wout_d = nc.declare_dram_parameter("wout", [1024, 1024], F32R, isOutput=False)
    w1_d = nc.declare_dram_parameter("w1", [1024, 4096], F32R, isOutput=False)
    w2_d = nc.declare_dram_parameter("w2", [4096, 1024], F32R, isOutput=False)
    ln1g_d = nc.declare_dram_parameter("ln1g", [1024], F32, isOutput=False)
    ln1b_d = nc.declare_dram_parameter("ln1b", [1024], F32, isOutput=False)
    ln2g_d = nc.declare_dram_parameter("ln2g", [1024], F32, isOutput=False)
    ln2b_d = nc.declare_dram_parameter("ln2b", [1024], F32, isOutput=False)
    bqkv_d = nc.declare_dram_parameter("bqkv", [3072], F32, isOutput=False)
    bout_d = nc.declare_dram_parameter("bout", [1024], F32, isOutput=False)
    b1_d = nc.declare_dram_parameter("b1", [4096], F32, isOutput=False)
    b2_d = nc.declare_dram_parameter("b2", [1024], F32, isOutput=False)
    mask_d = nc.declare_dram_parameter("mask", [4, 128, 256], F32, isOutput=False)
    out_d = nc.declare_dram_parameter("out", [512, 1024], F32, isOutput=True)

    with TileContext(nc) as tc:
        with tc.tile_pool(name="const", bufs=1) as pconst, \
             tc.tile_pool(name="glob", bufs=1) as glob, \
             tc.tile_pool(name="rot", bufs=2) as rot, \
             tc.tile_pool(name="small", bufs=4) as small:
            # ---- constants ----
            ident = pconst.tile([128, 128], F32, tag="ident", name="ident")
            make_identity(nc, ident[:])
            ln1g = pconst.tile([128, 8], F32, tag="ln1g", name="ln1g")
            ln1b = pconst.tile([128, 8], F32, tag="ln1b", name="ln1b")
            ln2g = pconst.tile([128, 8], F32, tag="ln2g", name="ln2g")
            ln2b = pconst.tile([128, 8], F32, tag="ln2b", name="ln2b")
            nc.sync.dma_start(out=ln1g[:], in_=ln1g_d.rearrange("(m p) -> p m", p=128))
            nc.sync.dma_start(out=ln1b[:], in_=ln1b_d.rearrange("(m p) -> p m", p=128))
            nc.sync.dma_start(out=ln2g[:], in_=ln2g_d.rearrange("(m p) -> p m", p=128))
            nc.sync.dma_start(out=ln2b[:], in_=ln2b_d.rearrange("(m p) -> p m", p=128))
            bq = pconst.tile([128, 8], F32, tag="bq", name="bq")
            bk = pconst.tile([128, 8], F32, tag="bk", name="bk")
            b1c = pconst.tile([128, 32], F32, tag="b1c", name="b1c")
            nc.sync.dma_start(out=bq[:], in_=bqkv_d[0:1024].rearrange("(m p) -> p m", p=128))
            nc.sync.dma_start(out=bk[:], in_=bqkv_d[1024:2048].rearrange("(m p) -> p m", p=128))
            nc.sync.dma_start(out=b1c[:], in_=b1_d.rearrange("(m p) -> p m", p=128))
            bv_bc = pconst.tile([128, 1024], F32, tag="bv_bc", name="bv_bc")
            bout_bc = pconst.tile([128, 1024], F32, tag="bout_bc", name="bout_bc")
            b2_bc = pconst.tile([128, 1024], F32, tag="b2_bc", name="b2_bc")
            brow = pconst.tile([1, 1024], F32, tag="brow", name="brow")
            for src, dst in ((bqkv_d[2048:3072], bv_bc), (bout_d[:], bout_bc),
                             (b2_d[:], b2_bc)):
                nc.sync.dma_start(out=brow[:], in_=src.rearrange("(o n) -> o n", o=1))
                nc.gpsimd.partition_broadcast(dst[:], brow[:])
            eps_t = pconst.tile([128, 1], F32, tag="eps", name="eps")
            nc.vector.memset(eps_t[:], 1e-5)
            ones16 = pconst.tile([128, 16], F32, tag="ones16", name="ones16")
            nc.vector.memset(ones16[:], 1.0)
            masks = pconst.tile([128, 4, 256], F32, tag="masks", name="masks")
            nc.sync.dma_start(out=masks[:], in_=mask_d.rearrange("t p q -> p t q"))

            # ---- globals across phases ----
            xnew = [glob.tile([128, 1024], F32, tag=f"xn{t}", name=f"xn{t}")
                    for t in range(4)]

            xgr = xg_d.rearrange("(t p) d -> t p d", p=128)

            for _rep in range(reps):
                with tc.tile_pool(name="attn", bufs=1) as attn:
                    # xg tiles: halo-prev tiles 0,1 (par0), 4,5 (par1);
                    # own tiles 2,3 (par0), 6,7 (par1)
                    xg_own = {}
                    oT = [attn.tile([128, 512], F32R, tag=f"oT{d}", name=f"oT{d}")
                          for d in range(8)]
                    for par in range(2):
                        with tc.tile_pool(name=f"pp{par}", bufs=1) as ppar:
                            hT = [ppar.tile([128, 512], F32R, tag=f"hT{d}", name=f"hT{d}")
                                  for d in range(8)]
                            qT = [ppar.tile([128, 256], F32R, tag=f"qT{m}", name=f"qT{m}")
                                  for m in range(8)]
                            kT = [ppar.tile([128, 512], F32R, tag=f"kT{m}", name=f"kT{m}")
                                  for m in range(8)]
                            Vt = [ppar.tile([128, 16, 65], F32R, tag=f"V{t}", name=f"V{t}")
                                  for t in range(4)]
                            # ---- LN1 + transpose ----
                            with tc.tile_pool(name="ppT", bufs=8, space="PSUM") as ppT:
                                pt4s = [ppT.tile([128, 4, 128], F32, tag="pt",
                                                 name=f"pt{d}") for d in range(8)]
                                for tt in range(4):
                                    t = par * 4 + tt
                                    if tt >= 2:
                                        xt = attn.tile([128, 1024], F32, tag=f"xg{t}",
                                                       name=f"xg{t}")
                                        xg_own[t] = xt
                                    else:
                                        xt = ppar.tile([128, 1024], F32, tag=f"xh{t}",
                                                       name=f"xh{t}")
                                    nc.sync.dma_start(out=xt[:], in_=xgr[t])
                                    h_t = rot.tile([128, 1024], F32, tag="h", name="h")
                                    stats = small.tile([128, 2, nc.vector.BN_STATS_DIM],
                                                       F32, tag="stats", name="stats")
                                    mv = small.tile([128, nc.vector.BN_AGGR_DIM], F32,
                                                    tag="mv", name="mv")
                                    rstd = small.tile([128, 1], F32, tag="rstd", name="rstd")
                                    for sg in range(2):
                                        nc.vector.bn_stats(out=stats[:, sg, :],
                                                           in_=xt[:, sg * 512:(sg + 1) * 512])
                                    nc.vector.bn_aggr(out=mv[:], in_=stats[:])
                                    nc.scalar.activation(out=rstd[:], in_=mv[:, 1:2],
                                                         func=AF.Sqrt, bias=eps_t[:], scale=1.0)
                                    nc.vector.reciprocal(out=rstd[:], in_=rstd[:])
                                    nc.vector.tensor_scalar(out=h_t[:], in0=xt[:],
                                                            scalar1=mv[:, 0:1], scalar2=rstd[:],
                                                            op0=OP.subtract, op1=OP.mult)
                                    for d in range(8):
                                        nc.tensor.matmul(pt4s[d][:, tt, :],
                                                         h_t[:, d * 128:(d + 1) * 128],
                                                         ident[:], is_transpose=True,
                                                         start=(tt == 0), stop=(tt == 3))
                                for d in range(8):
                                    nc.vector.tensor_scalar(
                                        out=hT[d][:],
                                        in0=pt4s[d][:].rearrange("p a b -> p (a b)"),
                                        scalar1=ln1g[:, d:d + 1],
                                        scalar2=ln1b[:, d:d + 1], op0=OP.mult, op1=OP.add)

                            # ---- QKV for this parity ----
                            with tc.tile_pool(name="pp512", bufs=4, space="PSUM") as pp512, \
                                 tc.tile_pool(name="wm", bufs=4) as wm, \
                                 tc.tile_pool(name="wrv", bufs=1) as wrv:
                                for m in range(8):
                                    w_t = wm.tile([128, 8, 128], F32R, tag="w", name="w")
                                    nc.sync.dma_start(
                                        out=w_t[:],
                                        in_=wqkv_d[:, m * 128:(m + 1) * 128]
                                            .rearrange("(k p) m -> p k m", p=128))
                                    ps = pp512.tile([128, 256], F32, tag="ps", name="psq")
                                    for k in range(8):
                                        nc.tensor.matmul(ps[:], w_t[:, k, :],
                                                         hT[k][:, 256:512],
                                                         start=(k == 0), stop=(k == 7))
                                    nc.vector.tensor_scalar(out=qT[m][:], in0=ps[:],
                                                            scalar1=bq[:, m:m + 1],
                                                            scalar2=SCALE,
                                                            op0=OP.add, op1=OP.mult)
                                for m in range(8):
                                    w_t = wm.tile([128, 8, 128], F32R, tag="w", name="w")
                                    nc.sync.dma_start(
                                        out=w_t[:],
                                        in_=wqkv_d[:, 1024 + m * 128:1024 + (m + 1) * 128]
                                            .rearrange("(k p) m -> p k m", p=128))
                                    ps = pp512.tile([128, 512], F32, tag="ps", name="psk")
                                    for k in range(8):
                                        nc.tensor.matmul(ps[:], w_t[:, k, :], hT[k][:],
                                                         start=(k == 0), stop=(k == 7))
                                    nc.vector.tensor_scalar_add(out=kT[m][:], in0=ps[:],
                                                                scalar1=bk[:, m:m + 1])
                                for npass in range(2):
                                    wv = []
                                    for k in range(8):
                                        wvk = wrv.tile([128, 512], F32R, tag=f"wv{k}",
                                                       name=f"wv{k}")
                                        nc.sync.dma_start(
                                            out=wvk[:],
                                            in_=wqkv_d[k * 128:(k + 1) * 128,
                                                       2048 + npass * 512:2048 + (npass + 1) * 512])
                                        wv.append(wvk)
                                    for tt in range(4):
                                        if npass == 0:
                                            nc.vector.tensor_copy(
                                                out=Vt[tt][:, :, 64:65],
                                                in_=ones16[:].rearrange("p (h e) -> p h e", e=1))
                                        ps = pp512.tile([128, 512], F32, tag="ps", name="psv")
                                        for k in range(8):
                                            nc.tensor.matmul(ps[:],
                                                             hT[k][:, tt * 128:(tt + 1) * 128],
                                                             wv[k][:],
                                                             start=(k == 0), stop=(k == 7))
                                        nc.vector.tensor_tensor(
                                            out=Vt[tt][:, npass * 8:(npass + 1) * 8, 0:64],
                                            in0=ps[:].rearrange("p (h e) -> p h e", h=8),
                                            in1=bv_bc[:, npass * 512:(npass + 1) * 512]
                                                .rearrange("p (h e) -> p h e", h=8),
                                            op=OP.add)

                            # ---- attention for this parity (2 kt per pass) ----
                            with tc.tile_pool(name="ppS", bufs=4, space="PSUM") as ppS, \
                                 tc.tile_pool(name="ppO", bufs=4, space="PSUM") as ppO:
                                for h in range(16):
                                    mt, po = h // 2, 64 * (h % 2)
                                    pso = ppO.tile([128, 256], F32, tag="pso", name="pso")
                                    for ktp in range(2):
                                        pss = ppS.tile([128, 2, 256], F32, tag="pss",
                                                       name="pss")
                                        for i in range(2):
                                            kt = 2 * ktp + i
                                            nc.tensor.matmul(
                                                pss[:, i, :],
                                                kT[mt][po:po + 64,
                                                       kt * 128:(kt + 1) * 128],
                                                qT[mt][po:po + 64, :],
                                                start=(i == 0), stop=(i == 1))
                                        nc.vector.tensor_tensor(
                                            out=pss[:], in0=pss[:],
                                            in1=masks[:, 2 * ktp:2 * ktp + 2, :],
                                            op=OP.add)
                                        expm = small.tile([128, 2, 256], F32R, tag="expm",
                                                          bufs=6, name="expm")
                                        nc.scalar.activation(out=expm[:], in_=pss[:],
                                                             func=AF.Exp)
                                        for i in range(2):
                                            nc.tensor.matmul(
                                                pso[0:65, :],
                                                Vt[2 * ktp + i][:, h, :],
                                                expm[:, i, :],
                                                start=(ktp == 0 and i == 0),
                                                stop=(ktp == 1 and i == 1))
                                    rec = small.tile([1, 256], F32, tag="rec", bufs=4,
                                                     name="rec")
                                    nc.vector.reciprocal(out=rec[:], in_=pso[64:65, :])
                                    rbc = small.tile([64, 256], F32, tag="rbc", bufs=4,
                                                     name="rbc")
                                    nc.gpsimd.partition_broadcast(rbc[:], rec[:])
                                    nc.vector.tensor_tensor(
                                        out=oT[mt][po:po + 64, par * 256:(par + 1) * 256],
                                        in0=pso[0:64, :], in1=rbc[:], op=OP.mult)

                    # ---- out-proj + residual (both parities) ----
                    with tc.tile_pool(name="pp8", bufs=1, space="PSUM") as pp8, \
                         tc.tile_pool(name="wr2", bufs=4) as wr2:
                        pso_ = [pp8.tile([128, 512], F32, tag=f"po{i}", name=f"po{i}")
                                for i in range(8)]
                        for k in range(8):
                            wo = wr2.tile([128, 1024], F32R, tag="wo", name="wo")
                            nc.sync.dma_start(out=wo[:], in_=wout_d[k * 128:(k + 1) * 128, :])
                            for tb in range(4):
                                for npass in range(2):
                                    nc.tensor.matmul(pso_[tb * 2 + npass][:],
                                                     oT[k][:, tb * 128:(tb + 1) * 128],
                                                     wo[:, npass * 512:(npass + 1) * 512],
                                                     start=(k == 0), stop=(k == 7))
                        for tb in range(4):
                            xob = small.tile([128, 1024], F32, tag="xob", bufs=2, name="xob")
                            xg_o = xg_own[(tb // 2) * 4 + 2 + tb % 2]
                            nc.gpsimd.tensor_add(out=xob[:], in0=xg_o[:], in1=bout_bc[:])
                            for npass in range(2):
                                nc.vector.tensor_tensor(
                                    out=xnew[tb][:, npass * 512:(npass + 1) * 512],
                                    in0=pso_[tb * 2 + npass][:],
                                    in1=xob[:, npass * 512:(npass + 1) * 512], op=OP.add)

                # ---- FFN ----
                with tc.tile_pool(name="ffn", bufs=1) as ffn:
                    h2T = [ffn.tile([128, 512], F32R, tag=f"h2T{d}", name=f"h2T{d}")
                           for d in range(8)]
                    fT = [ffn.tile([128, 512], F32R, tag=f"fT{m}", name=f"fT{m}")
                          for m in range(32)]
                    with tc.tile_pool(name="ppT2", bufs=8, space="PSUM") as ppT2:
                        pt4s = [ppT2.tile([128, 4, 128], F32, tag="pt",
                                          name=f"pt2_{d}") for d in range(8)]
                        for t in range(4):
                            h2 = rot.tile([128, 1024], F32, tag="h", name="h2")
                            stats = small.tile([128, 2, nc.vector.BN_STATS_DIM], F32,
                                               tag="stats", name="stats")
                            mv = small.tile([128, nc.vector.BN_AGGR_DIM], F32, tag="mv",
                                            name="mv")
                            rstd = small.tile([128, 1], F32, tag="rstd", name="rstd")
                            for sg in range(2):
                                nc.vector.bn_stats(out=stats[:, sg, :],
                                                   in_=xnew[t][:, sg * 512:(sg + 1) * 512])
                            nc.vector.bn_aggr(out=mv[:], in_=stats[:])
                            nc.scalar.activation(out=rstd[:], in_=mv[:, 1:2], func=AF.Sqrt,
                                                 bias=eps_t[:], scale=1.0)
                            nc.vector.reciprocal(out=rstd[:], in_=rstd[:])
                            nc.vector.tensor_scalar(out=h2[:], in0=xnew[t][:],
                                                    scalar1=mv[:, 0:1], scalar2=rstd[:],
                                                    op0=OP.subtract, op1=OP.mult)
                            # xnew += b2 (residual base for FFN2, in place)
                            nc.gpsimd.tensor_add(out=xnew[t][:], in0=xnew[t][:],
                                                  in1=b2_bc[:])
                            for d in range(8):
                                nc.tensor.matmul(pt4s[d][:, t, :],
                                                 h2[:, d * 128:(d + 1) * 128],
                                                 ident[:], is_transpose=True,
                                                 start=(t == 0), stop=(t == 3))
                        for d in range(8):
                            nc.vector.tensor_scalar(
                                out=h2T[d][:],
                                in0=pt4s[d][:].rearrange("p a b -> p (a b)"),
                                scalar1=ln2g[:, d:d + 1], scalar2=ln2b[:, d:d + 1],
                                op0=OP.mult, op1=OP.add)

                    with tc.tile_pool(name="pp512b", bufs=4, space="PSUM") as pp512b, \
                         tc.tile_pool(name="wm2", bufs=4) as wm2:
                        for m in range(32):
                            w_t = wm2.tile([128, 8, 128], F32R, tag="w", name="w")
                            nc.sync.dma_start(
                                out=w_t[:],
                                in_=w1_d[:, m * 128:(m + 1) * 128]
                                    .rearrange("(k p) m -> p k m", p=128))
                            ps = pp512b.tile([128, 512], F32, tag="ps", name="ps")
                            for k in range(8):
                                nc.tensor.matmul(ps[:], w_t[:, k, :], h2T[k][:],
                                                 start=(k == 0), stop=(k == 7))
                            nc.scalar.activation(out=fT[m][:], in_=ps[:], func=AF.Gelu,
                                                 bias=b1c[:, m:m + 1], scale=1.0)

                    with tc.tile_pool(name="pp8b", bufs=1, space="PSUM") as pp8b, \
                         tc.tile_pool(name="wr3", bufs=4) as wr3:
                        psf = [pp8b.tile([128, 512], F32, tag=f"pf{i}", name=f"pf{i}")
                               for i in range(8)]
                        for k in range(32):
                            w2t = wr3.tile([128, 1024], F32R, tag="w2", name="w2")
                            nc.sync.dma_start(out=w2t[:], in_=w2_d[k * 128:(k + 1) * 128, :])
                            for tb in range(4):
                                for npass in range(2):
                                    nc.tensor.matmul(psf[tb * 2 + npass][:],
                                                     fT[k][:, tb * 128:(tb + 1) * 128],
                                                     w2t[:, npass * 512:(npass + 1) * 512],
                                                     start=(k == 0), stop=(k == 31))
                        for tb in range(4):
                            for npass in range(2):
                                nc.vector.tensor_tensor(
                                    out=xnew[tb][:, npass * 512:(npass + 1) * 512],
                                    in0=psf[tb * 2 + npass][:],
                                    in1=xnew[tb][:, npass * 512:(npass + 1) * 512],
                                    op=OP.add)

            for t in range(4):
                nc.sync.dma_start(out=out_d.rearrange("(t p) d -> t p d", p=128)[t],
                                  in_=xnew[t][:])

    nc.compile()
    return nc


def make_masks():
    """Per-chunk additive masks [4][4, 128, 256]."""
    out = []
    for c in range(4):
        m = np.full((4, 128, 256), NEG, np.float32)
        for t in range(4):
            kk = np.arange(128)[:, None]
            q = np.arange(256)[None, :]
            Qg = c * 256 + q
            Kg = c * 256 - 256 + t * 128 + kk
            keep = (Kg >= 0) & (Qg - Kg >= 0) & (Qg - Kg <= 256)
            m[t][keep] = 0.0
        out.append(m)
    return out


def make_in_maps(inputs):
    x = np.asarray(inputs["x"], np.float32)
    masks = make_masks()
    common = {
        "wqkv": np.ascontiguousarray(np.asarray(inputs["Wqkv"], np.float32)),
        "wout": np.ascontiguousarray(np.asarray(inputs["Wout"], np.float32)),
        "w1": np.ascontiguousarray(np.asarray(inputs["W1"], np.float32)),
        "w2": np.ascontiguousarray(np.asarray(inputs["W2"], np.float32)),
        "ln1g": np.asarray(inputs["ln1_g"], np.float32),
        "ln1b": np.asarray(inputs["ln1_b"], np.float32),
        "ln2g": np.asarray(inputs["ln2_g"], np.float32),
        "ln2b": np.asarray(inputs["ln2_b"], np.float32),
        "bqkv": np.asarray(inputs["bqkv"], np.float32),
        "bout": np.asarray(inputs["bout"], np.float32),
        "b1": np.asarray(inputs["b1"], np.float32),
        "b2": np.asarray(inputs["b2"], np.float32),
    }
    in_maps = []
    for core in range(8):
        b, c = core // 4, core % 4
        xg = np.zeros((1024, 1024), np.float32)
        for par in range(2):
            i0, i1 = c * 256 - 256, c * 256 + 256
            ii = np.arange(max(i0, 0), i1)
            xg[par * 512 + (ii - i0), :] = x[b, 2 * ii + par, :]
        in_maps.append({**common, "xg": xg, "mask": masks[c]})
    return in_maps


def assemble(results):
    out = np.zeros((B, L, D), np.float32)
    for core in range(8):
        b, c = core // 4, core % 4
        o = results[core]["out"]
        for par in range(2):
            ii = np.arange(c * 256, (c + 1) * 256)
            out[b, 2 * ii + par, :] = o[par * 256:(par + 1) * 256, :]
    return out



_CACHE = {}


def kernel(**inputs):
    """Full-input entry point: shards across 8 NeuronCores, runs the Bass
    kernel SPMD, gathers the full [B, L, D] float32 output."""
    if "nc" not in _CACHE:
        _CACHE["nc"] = build()
    nc = _CACHE["nc"]
    in_maps = make_in_maps(inputs)
    res = run_bass_kernel_spmd(nc, in_maps, list(range(8)))
    return assemble(res.results)

